# revision 52
# baseline (speedup 1.0000x reference)
"""Equivariant MPNN layer as a Bass/Tile kernel for TRN2.

Strategy (per problem nn_EquivariantMPNNLayer):
  - Edges are sorted by destination grid cell (j) on the host and sharded
    across cores by contiguous 128-segment blocks (G segs / NCORES per core).
  - The host pre-gathers per-edge node embeddings (nemb[i_e] + c, bf16,
    H-major, slot order) so the device needs NO random-access gather: the
    per-node message term D[i] = nemb[i]@Wm1_top + bmix is realized as one
    PSUM-accumulate matmul per tile (lhsT=nemb tile, rhs=Wm1_top), with the
    bias folded in via c = solve(Wm1_top^T, bmix) added host-side.
  - Geometry: host premultiplies z[3b+c] = R[b,c]*gp[c] and rp = R np, ships
    zin = [z; rp] (12 rows, bf16); pre1 = W1big^T zin + be1 in one K=12
    matmul with W1big = [repeat(We1,3); -We1].
  - msg' = silu(silu(pre1) @ Wf + D[i]) with Wf = We2 @ Wm1_bot (Wm2 folded
    out of the edge path).
  - Segment means via one-hot matmul into PSUM per 128-seg block
    (lhsT=msg tile, rhs=one-hot -> S^T directly), with the per-edge weight
    1/max(cnt,1) folded into the one-hot values (host-sent).
  - Post: mean' = S' @ Wm2 + bm2*[cnt>0], out = MLP_u(mean'), in [h, seg]
    layout with stationary weights; gate row [cnt>0] comes from the host.
  - Edge-path compute is bf16 (PE 1 cyc/row); the loop body is software-
    pipelined with a 4-deep stage skew so PE/ACT/DVE run back-to-back.
Outputs are [128, SEGS_PER_CORE] (transposed) per core; host reassembles.
"""

import math
from contextlib import ExitStack

import numpy as np
import ml_dtypes

import concourse.bass as bass
import concourse.tile as tile
from concourse import bacc, mybir

F32 = mybir.dt.float32
BF16 = mybir.dt.bfloat16
I16 = mybir.dt.int16
AOT = mybir.AluOpType

H = 128
P = 128
BLK = 128          # segments per psum block
GRP = 512          # edges per pipeline iteration (4 tiles)
GRP_TILES = 4
CHUNK_TILES = 32   # tiles per dma chunk (4096 edges)
CHUNK_E = CHUNK_TILES * P


class Cfg:
    def __init__(self, N, G, E, B, ncores, T, reps=1, use_bf16=True,
                 loop_k=1):
        self.loop_k = loop_k
        self.N, self.G, self.E, self.B = N, G, E, B
        self.ncores = ncores
        assert G % (ncores * BLK) == 0
        self.segs_core = G // ncores          # segments per core
        self.nblk = self.segs_core // BLK     # psum blocks per core
        self.T = T                            # tiles (of 128 edges) per block
        self.ntiles = self.nblk * T           # tiles per core
        assert self.ntiles % CHUNK_TILES == 0
        self.e_pad = self.ntiles * P          # padded edges per core
        self.nchunks = self.ntiles // CHUNK_TILES
        self.niters = self.ntiles // GRP_TILES
        self.reps = reps
        self.use_bf16 = use_bf16


def build_program(cfg: Cfg):
    """Build the SPMD per-core Bass program. Returns compiled nc."""
    nc = bacc.Bacc("TRN2", target_bir_lowering=False, debug=False,
                   num_devices=cfg.ncores)
    dt_e = BF16 if cfg.use_bf16 else F32   # dtype for edge-path operands

    # ---------------- I/O ----------------
    def din(name, shape, dt=F32):
        return nc.dram_tensor(name, shape, dt, kind="ExternalInput").ap()

    Wm1e = din("Wm1e", [H, H], dt_e)                 # Wm1[:H] in edge dtype
    W1big = din("W1big", [12, H], dt_e)              # [repeat(We1,3); -We1]
    be1c = din("be1c", [H, 1])
    Wf = din("Wf", [H, H], dt_e)                     # We2 @ Wm1_bot
    IOTA = din("IOTA", [P, P], dt_e)                 # IOTA[e,s] = s
    Wm2 = din("Wm2", [H, H])
    bm2r = din("bm2r", [1, H])
    Wu1 = din("Wu1", [H, H])
    bu1c = din("bu1c", [H, 1])
    Wu2 = din("Wu2", [H, H])
    bu2c = din("bu2c", [H, 1])

    if cfg.loop_k > 1:
        din("ktag", [1, cfg.loop_k])                 # shape tag to defeat HLO cache
    nbe = din("nbe", [P, cfg.e_pad], dt_e)           # (nemb[i_e]+c)^T, slot order
    zin = din("zin", [12, cfg.e_pad], dt_e)          # rows 0:9 R*gp, 9:12 R@np
    segf = din("segf", [P, cfg.ntiles], F32)         # per-edge seg-in-block (or -1)
    winv = din("winv", [P, cfg.ntiles], F32)         # per-edge 1/max(cnt,1)
    grow = din("grow", [1, cfg.segs_core])           # per-seg gate [cnt>0]

    outT = nc.dram_tensor("outT", [H, cfg.segs_core], F32,
                          kind="ExternalOutput").ap()

    with tile.TileContext(nc) as tc, ExitStack() as ctx:
        ep = ctx.enter_context  # shorthand

        consts = ep(tc.tile_pool(name="consts", bufs=1))
        gpool = ep(tc.tile_pool(name="gpool", bufs=4))
        zpool = ep(tc.tile_pool(name="zpool", bufs=4))
        hpool = ep(tc.tile_pool(name="hpool", bufs=4))
        mpool = ep(tc.tile_pool(name="mpool", bufs=4))
        apool = ep(tc.tile_pool(name="apool", bufs=12))
        spool = ep(tc.tile_pool(name="spool", bufs=1))
        postp = ep(tc.tile_pool(name="postp", bufs=2))
        # PSUM: pp 1 bank x3 + pm 1x3 + psS 1x2 = 8 banks
        pspp = ep(tc.tile_pool(name="pspp", bufs=3, space="PSUM"))
        pspm = ep(tc.tile_pool(name="pspm", bufs=3, space="PSUM"))
        psS = ep(tc.tile_pool(name="psS", bufs=2, space="PSUM"))

        # ---- load constants into SBUF ----
        def csb(ap_in, shape, dt=F32, tag=None):
            t = consts.tile(shape, dt, tag=tag or ap_in.tensor.name)
            nc.sync.dma_start(t[:], ap_in)
            return t

        Wm1e_sb = csb(Wm1e, [H, H], dt_e)
        W1big_sb = csb(W1big, [12, H], dt_e)
        be1_sb = csb(be1c, [H, 1])
        Wf_sb = csb(Wf, [H, H], dt_e)
        IOTA_sb = csb(IOTA, [P, P], dt_e)
        Wm2_sb = csb(Wm2, [H, H])
        bm2_sb = csb(bm2r, [1, H])
        Wu1_sb = csb(Wu1, [H, H])
        bu1_sb = csb(bu1c, [H, 1])
        Wu2_sb = csb(Wu2, [H, H])
        bu2_sb = csb(bu2c, [H, 1])
        segf_sb = consts.tile([P, cfg.ntiles], F32, tag="segf")
        nc.sync.dma_start(segf_sb[:], segf)
        winv_sb = consts.tile([P, cfg.ntiles], F32, tag="winv")
        nc.sync.dma_start(winv_sb[:], winv)

        # ---- main loop (repeatable for timing) ----
        loop_cm = tc.For_i(0, cfg.loop_k, 1) if cfg.loop_k > 1 else None
        if loop_cm is not None:
            ctx.enter_context(loop_cm)
        skip = getattr(cfg, "skip", frozenset())  # timing-variant knobs
        for rep in range(cfg.reps):
            ST_all = spool.tile([H, cfg.segs_core], F32, tag="ST")
            if "seg" in skip:
                nc.vector.memset(ST_all[:, 0:1], 0)

            # live pipeline state, keyed by iteration index
            pps, h1s, pms, msgps, nbcs, zcs = {}, {}, {}, {}, {}, {}
            ps_blk = [None]

            def start_chunk(c):
                if c >= cfg.nchunks:
                    return
                e0 = c * CHUNK_E
                zc = zpool.tile([12, CHUNK_E], dt_e, tag="zc")
                if "zdma" not in skip:
                    nc.sync.dma_start(zc[:], zin[:, e0:e0 + CHUNK_E])
                else:
                    nc.vector.memset(zc[:, 0:1], 0)
                zcs[c] = zc
                nbc = gpool.tile([P, CHUNK_E], dt_e, tag="nbc")
                if "gather" not in skip:
                    nc.sync.dma_start(nbc[:], nbe[:, e0:e0 + CHUNK_E])
                else:
                    nc.vector.memset(nbc[:, 0:1], 0)
                nbcs[c] = nbc

            def emit_mm1(i):
                zc = zcs[i * GRP // CHUNK_E]
                o = (i * GRP) % CHUNK_E
                pp = pspp.tile([H, GRP], F32, tag="pp")
                if "mm1" not in skip:
                    nc.tensor.matmul(out=pp[:], lhsT=W1big_sb[:],
                                     rhs=zc[:, o:o + GRP],
                                     start=True, stop=True)
                else:
                    nc.vector.memset(pp[:, 0:1], 0)
                pps[i] = pp

            def emit_silu1(i):
                h1 = hpool.tile([H, GRP], dt_e, tag="h1")
                if "silu" not in skip:
                    nc.scalar.activation(out=h1[:], in_=pps.pop(i)[:],
                                         func=mybir.ActivationFunctionType.Silu,
                                         bias=be1_sb[:])
                else:
                    nc.vector.memset(h1[:, 0:1], 0)
                h1s[i] = h1

            def emit_mm2(i):
                h1 = h1s.pop(i)
                c = i * GRP // CHUNK_E
                nbc = nbcs[c]
                o = (i * GRP) % CHUNK_E
                pm = pspm.tile([P, GRP], F32, tag="pm")
                if "mm2" not in skip:
                    for t in range(GRP_TILES):
                        co = t * P
                        nc.tensor.matmul(out=pm[:, co:co + P],
                                         lhsT=h1[:, co:co + P], rhs=Wf_sb[:],
                                         start=True, stop=False)
                        nc.tensor.matmul(out=pm[:, co:co + P],
                                         lhsT=nbc[:, o + co:o + co + P],
                                         rhs=Wm1e_sb[:],
                                         start=False, stop=True)
                else:
                    nc.vector.memset(pm[:, 0:1], 0)
                pms[i] = pm

            def emit_silu2(i):
                msgp = mpool.tile([P, GRP], dt_e, tag="msgp")
                if "silu" not in skip:
                    nc.scalar.activation(out=msgp[:], in_=pms.pop(i)[:],
                                         func=mybir.ActivationFunctionType.Silu)
                else:
                    nc.vector.memset(msgp[:, 0:1], 0)
                msgps[i] = msgp

            def emit_seg(i):
                msgp = msgps.pop(i)
                if "seg" in skip:
                    return
                for t in range(GRP_TILES):
                    gt = i * GRP_TILES + t
                    blk = gt // cfg.T
                    tin = gt % cfg.T
                    at = apool.tile([P, P], dt_e, tag="at")
                    nc.vector.tensor_scalar(
                        out=at[:], in0=IOTA_sb[:],
                        scalar1=segf_sb[:, gt:gt + 1],
                        scalar2=winv_sb[:, gt:gt + 1],
                        op0=AOT.is_equal, op1=AOT.mult)
                    if tin == 0:
                        # S^T block accumulator: lhsT=msg tile, rhs=at
                        ps_blk[0] = psS.tile([H, BLK], F32, tag="psS",
                                             name="ps_blk")
                    nc.tensor.matmul(out=ps_blk[0][:],
                                     lhsT=msgp[:, t * P:t * P + P],
                                     rhs=at[:],
                                     start=(tin == 0), stop=(tin == cfg.T - 1))
                    if tin == cfg.T - 1:
                        nc.vector.tensor_copy(
                            out=ST_all[:, blk * BLK:(blk + 1) * BLK],
                            in_=ps_blk[0][:])

            for i in range(cfg.niters + 4):
                if i < cfg.niters:
                    if i == 0:
                        start_chunk(0)
                        start_chunk(1)
                        start_chunk(2)
                    elif i % (CHUNK_E // GRP) == 0:
                        start_chunk(i * GRP // CHUNK_E + 2)
                    emit_mm1(i)
                if 1 <= i and i - 1 < cfg.niters:
                    emit_silu1(i - 1)
                if 2 <= i and i - 2 < cfg.niters:
                    emit_mm2(i - 2)
                if 3 <= i and i - 3 < cfg.niters:
                    emit_silu2(i - 3)
                if 4 <= i and i - 4 < cfg.niters:
                    emit_seg(i - 4)

            if "post" not in skip:
                _post_stage(nc, cfg, ST_all, grow, Wm2_sb, bm2_sb, Wu1_sb,
                            bu1_sb, Wu2_sb, bu2_sb, outT, postp, pspp, pspm)

    nc.compile()
    return nc


def _post_stage(nc, cfg, ST_all, grow, Wm2_sb, bm2_sb, Wu1_sb, bu1_sb,
                Wu2_sb, bu2_sb, outT, postp, pspp, pspm):
    nseg_chunks = math.ceil(cfg.segs_core / 512)
    for u in range(nseg_chunks):
        s0 = u * 512
        w = min(512, cfg.segs_core - s0)
        gch = postp.tile([1, 512], F32, tag="gch")
        nc.sync.dma_start(gch[0:1, :w], grow[0:1, s0:s0 + w])
        pmm = pspp.tile([H, 512], F32, tag="pp")
        nc.tensor.matmul(out=pmm[:, :w], lhsT=Wm2_sb[:],
                         rhs=ST_all[:, s0:s0 + w], start=True, stop=False)
        nc.tensor.matmul(out=pmm[:, :w], lhsT=bm2_sb[:],
                         rhs=gch[0:1, :w], start=False, stop=True)
        mean = postp.tile([H, 512], F32, tag="mean")
        nc.vector.tensor_copy(out=mean[:, :w], in_=pmm[:, :w])
        pu = pspm.tile([H, 512], F32, tag="pm")
        nc.tensor.matmul(out=pu[:, :w], lhsT=Wu1_sb[:], rhs=mean[:, :w],
                         start=True, stop=True)
        hu = postp.tile([H, 512], F32, tag="hu")
        nc.scalar.activation(out=hu[:, :w], in_=pu[:, :w],
                             func=mybir.ActivationFunctionType.Silu,
                             bias=bu1_sb[:])
        po = pspp.tile([H, 512], F32, tag="pp")
        nc.tensor.matmul(out=po[:, :w], lhsT=Wu2_sb[:], rhs=hu[:, :w],
                         start=True, stop=True)
        ot = postp.tile([H, 512], F32, tag="ot")
        nc.scalar.activation(out=ot[:, :w], in_=po[:, :w],
                             func=mybir.ActivationFunctionType.Identity,
                             bias=bu2_sb[:])
        nc.sync.dma_start(outT[:, s0:s0 + w], ot[:, :w])


# ======================= host preprocessing =======================

def silu_np(x):
    return x / (1.0 + np.exp(-x))


def host_prep(inputs, ncores, use_bf16=True, t_override=None):
    """Returns (cfg, list of per-core in_maps, const row for node outputs)."""
    nemb = np.asarray(inputs["node_embedding"], np.float32)
    npos = np.asarray(inputs["node_pos"], np.float32)
    gpos = np.asarray(inputs["grid_pos"], np.float32)
    eidx = np.asarray(inputs["edge_index"], np.int64)
    frames = np.asarray(inputs["equi_frames"], np.float32)
    batch = np.asarray(inputs["batch"], np.int64)
    We1 = np.asarray(inputs["We1"], np.float32); be1 = np.asarray(inputs["be1"], np.float32)
    We2 = np.asarray(inputs["We2"], np.float32); be2 = np.asarray(inputs["be2"], np.float32)
    Wm1 = np.asarray(inputs["Wm1"], np.float32); bm1 = np.asarray(inputs["bm1"], np.float32)
    Wm2 = np.asarray(inputs["Wm2"], np.float32); bm2 = np.asarray(inputs["bm2"], np.float32)
    Wu1 = np.asarray(inputs["Wu1"], np.float32); bu1 = np.asarray(inputs["bu1"], np.float32)
    Wu2 = np.asarray(inputs["Wu2"], np.float32); bu2 = np.asarray(inputs["bu2"], np.float32)

    N, Hh = nemb.shape
    G = gpos.shape[0]
    E = eidx.shape[1]
    B = frames.shape[0]
    assert Hh == H

    i_all = eidx[0]
    jg_all = eidx[1] - N
    order = np.argsort(jg_all, kind="stable")
    jg_s = jg_all[order]
    i_s = i_all[order]

    gb = jg_s // BLK
    counts_blk = np.bincount(gb, minlength=G // BLK)
    T = int(math.ceil(counts_blk.max() / P))
    if t_override:
        T = max(T, t_override)
    nblk = (G // ncores) // BLK
    # tiles-per-core must divide into CHUNK_TILES-tile chunks
    k = CHUNK_TILES // math.gcd(nblk, CHUNK_TILES)
    T = int(math.ceil(T / k)) * k
    cfg = Cfg(N, G, E, B, ncores, T, use_bf16=use_bf16)

    # destination slot for each sorted edge
    starts = np.zeros(G // BLK + 1, np.int64)
    starts[1:] = np.cumsum(counts_blk)
    rank = np.arange(E) - starts[gb]
    core_e = gb // cfg.nblk
    b_local = gb % cfg.nblk
    slot = b_local * (T * P) + rank

    # per-edge host gathers (pure data movement + O(N)/O(B) math)
    R_flat = frames.reshape(B, 9)
    b_e = batch[i_s]
    z_r = R_flat[b_e].T.astype(np.float32)               # [9, E]
    gp_e = gpos[jg_s].T.astype(np.float32)               # [3, E]
    gp_rep = np.tile(gp_e, (3, 1))                       # [9, E]
    z_pre = z_r * gp_rep                                 # [9, E] premultiplied
    rp_node = np.einsum("nab,nb->na", frames[batch], npos).astype(np.float32)
    rp_e = rp_node[i_s].T                                # [3, E]

    ecount = np.bincount(jg_all, minlength=G).astype(np.float32)
    winv_e = 1.0 / np.maximum(ecount[jg_s], 1.0)         # per sorted edge

    dt_g = ml_dtypes.bfloat16 if use_bf16 else np.float32

    # fold bmix = bm1 + be2@Wm1_bot into the shipped embeddings:
    # (nemb + c) @ Wm1_top == nemb @ Wm1_top + bmix  with  c @ Wm1_top = bmix
    bmix = bm1 + be2 @ Wm1[H:]
    Wtop = Wm1[:H]
    if np.abs(bmix).max() > 0:
        c = np.linalg.lstsq(Wtop.T, bmix, rcond=None)[0]
        assert np.abs(c @ Wtop - bmix).max() < 1e-4 * (1 + np.abs(bmix).max())
    else:
        c = np.zeros_like(bmix)
    nemb_c = (nemb + c[None, :]).astype(dt_g)            # [N, H]

    shared = {
        "Wm1e": np.ascontiguousarray(Wtop).astype(dt_g),
        "W1big": np.ascontiguousarray(
            np.concatenate([np.repeat(We1, 3, axis=0), -We1], 0)).astype(dt_g),
        "be1c": np.ascontiguousarray(be1[:, None]),
        "Wf": np.ascontiguousarray(We2 @ Wm1[H:]).astype(dt_g),
        "IOTA": np.ascontiguousarray(
            np.tile(np.arange(P, dtype=np.float32)[None, :], (P, 1))).astype(dt_g),
        "Wm2": np.ascontiguousarray(Wm2),
        "bm2r": np.ascontiguousarray(bm2[None, :]),
        "Wu1": np.ascontiguousarray(Wu1),
        "bu1c": np.ascontiguousarray(bu1[:, None]),
        "Wu2": np.ascontiguousarray(Wu2),
        "bu2c": np.ascontiguousarray(bu2[:, None]),
    }

    in_maps = []
    for c_id in range(ncores):
        sel = core_e == c_id
        sl = slot[sel]
        nbe = np.zeros((cfg.e_pad, H), dt_g)
        nbe[sl] = nemb_c[i_s[sel]]
        zin = np.zeros((12, cfg.e_pad), np.float32)
        zin[0:9, sl] = z_pre[:, sel]
        zin[9:12, sl] = rp_e[:, sel]
        segf_f = np.full(cfg.e_pad, -1.0, np.float32)
        segf_f[sl] = (jg_s[sel] % BLK).astype(np.float32)
        winv_f = np.ones(cfg.e_pad, np.float32)
        winv_f[sl] = winv_e[sel]

        segf_t = segf_f.reshape(cfg.ntiles, P).T            # [128, ntiles]
        winv_t = winv_f.reshape(cfg.ntiles, P).T            # [128, ntiles]
        grow_c = (ecount[c_id * cfg.segs_core:(c_id + 1) * cfg.segs_core] > 0
                  ).astype(np.float32)[None, :]             # [1, segs]

        m = dict(shared)
        m["nbe"] = np.ascontiguousarray(nbe.T)              # [128, e_pad]
        m["zin"] = np.ascontiguousarray(zin).astype(dt_g)
        m["segf"] = np.ascontiguousarray(segf_t)
        m["winv"] = np.ascontiguousarray(winv_t)
        m["grow"] = np.ascontiguousarray(grow_c)
        in_maps.append(m)

    const_row = silu_np(bu1) @ Wu2 + bu2
    return cfg, in_maps, const_row


def assemble_output(cfg, results, const_row, N, G):
    out = np.empty((N + G, H), np.float32)
    out[:N] = const_row[None, :]
    for c in range(cfg.ncores):
        out[N + c * cfg.segs_core: N + (c + 1) * cfg.segs_core] = \
            results[c]["outT"].T
    return out


# ======================= top-level kernel entry =======================

_PROGRAM_CACHE = {}

NCORES = 8
USE_BF16 = True


def kernel(**inputs):
    """Full-input entry point: shards edges by destination grid cell across
    8 NeuronCores, runs the Bass/Tile program, reassembles the full output."""
    from concourse.bass_utils import run_bass_kernel_spmd

    cfg, in_maps, const_row = host_prep(inputs, NCORES, use_bf16=USE_BF16)
    key = (cfg.T, cfg.use_bf16)
    if key not in _PROGRAM_CACHE:
        _PROGRAM_CACHE[key] = build_program(cfg)
    nc = _PROGRAM_CACHE[key]
    res = run_bass_kernel_spmd(nc, in_maps, core_ids=list(range(NCORES)))
    N = inputs["node_pos"].shape[0]
    G = inputs["grid_pos"].shape[0]
    return assemble_output(cfg, res.results, const_row, N, G)
